# revision 5
# baseline (speedup 1.0000x reference)
"""BitNet MLP (nn_BitNetMLP) Trainium2 kernel — 8-core tensor-parallel over
the intermediate dimension I.

Math (reference):
  xq    = int4_absmean_quant(x)          per-token over H
  gate  = xq @ (ternary(w_gate)*wm_g).T
  up    = xq @ (ternary(w_up)*wm_u).T
  inter = int8_absmax_quant(up * relu(gate)^2)   per-token over I
  out   = inter @ (ternary(w_down)*wm_d).T

All quantized values are small integers; matmuls run with integer-valued
fp8/bf16 operands and fp32 PSUM accumulation -> exact integer arithmetic.
Scales (beta_t, w_mean, gamma_t) fold into per-token scalars applied on the
ScalarE during PSUM evacuation. Rounding = float32 magic-number trick (RNE,
matches jnp.round); clip applied post-round in magic space.

Sharding: each core holds I/8 rows of w_gate/w_up, I/8 cols of w_down, and
full x. Collectives: AllReduce(add) of |w| sums (w_mean), AllReduce(max) of
per-token gamma partials, ReduceScatter(add) of [T, H] output partials.
"""

import numpy as np

N_CORES = 8
B, S = 2, 2048
H, I = 4096, 11008
T = B * S                    # 4096 tokens
IC = I // N_CORES            # 1376 intermediate per core
ICQ = IC // 4                # 344 (mm1 PSUM quarter, 1 bank)
NSLAB = 11                   # ceil(1376/128) i-slabs for mm2
ICP = NSLAB * 128            # 1408 padded
TSUP = 512                   # tokens per super-block
NSUP = T // TSUP             # 8
TT = 128                     # tokens per tile
NT = T // TT                 # 32 t_tiles
NTS = TSUP // TT             # 4 t_tiles per super
HB = H // 128                # 32 h-blocks (mm1 contraction k-tiles)
HH = H // 2                  # 2048 (x processed in H-halves)
MAGIC = 12582912.0           # 1.5 * 2^23: float32 round-to-nearest-int trick
EPS = 1e-5
SQRT7 = float(np.sqrt(7.0))

_cache = {}


def _build(debug=False):
    import contextlib
    import concourse.mybir as mybir
    import concourse.tile as tile
    from concourse import bacc, bass_isa

    dt = mybir.dt
    Alu = mybir.AluOpType
    Act = mybir.ActivationFunctionType

    nc = bacc.Bacc("TRN2", target_bir_lowering=False, debug=False,
                   num_devices=N_CORES)

    x_in = nc.dram_tensor("x", [T, H], dt.float32, kind="ExternalInput")
    wg_in = nc.dram_tensor("wg", [IC, H], dt.float32, kind="ExternalInput")
    wu_in = nc.dram_tensor("wu", [IC, H], dt.float32, kind="ExternalInput")
    wd_in = nc.dram_tensor("wd", [H, IC], dt.float32, kind="ExternalInput")
    sc_in = nc.dram_tensor("scales", [1, 3], dt.float32, kind="ExternalInput")
    out_ext = nc.dram_tensor("out_rs", [NSUP, TSUP // N_CORES, H], dt.float32,
                             kind="ExternalOutput")
    if debug:
        dbg = {
            "dbg_beta": nc.dram_tensor("dbg_beta", [128, NT], dt.float32, kind="ExternalOutput"),
            "dbg_gam": nc.dram_tensor("dbg_gam", [128, NT], dt.float32, kind="ExternalOutput"),
            "dbg_s2": nc.dram_tensor("dbg_s2", [128, NT], dt.float32, kind="ExternalOutput"),
            "dbg_Dt": nc.dram_tensor("dbg_Dt", [128, NT], dt.float32, kind="ExternalOutput"),
            "dbg_wm": nc.dram_tensor("dbg_wm", [1, 3], dt.float32, kind="ExternalOutput"),
            "dbg_xq": nc.dram_tensor("dbg_xq", [TSUP, H], dt.bfloat16, kind="ExternalOutput"),
            "dbg_wqg": nc.dram_tensor("dbg_wqg", [IC, H], dt.bfloat16, kind="ExternalOutput"),
            "dbg_p": nc.dram_tensor("dbg_p", [TT, IC], dt.float32, kind="ExternalOutput"),
            "dbg_iq": nc.dram_tensor("dbg_iq", [TSUP, ICP], dt.bfloat16, kind="ExternalOutput"),
            "dbg_part": nc.dram_tensor("dbg_part", [TSUP, H], dt.float32, kind="ExternalOutput"),
        }

    RG = [list(range(N_CORES))]

    def row_tiles(rows):
        out, r0 = [], 0
        while r0 < rows:
            out.append((r0, min(128, rows - r0)))
            r0 += 128
        return out

    with tile.TileContext(nc) as tc:
        ctx = contextlib.ExitStack()
        with ctx:
            dram = ctx.enter_context(tc.tile_pool(name="dram", bufs=1, space="DRAM"))
            wqg_d = dram.tile([IC, H], dt.bfloat16, tag="wqg_d")
            wqu_d = dram.tile([IC, H], dt.bfloat16, tag="wqu_d")
            wqd_d = dram.tile([H, IC], dt.bfloat16, tag="wqd_d")
            xq_d = [dram.tile([TSUP, H], dt.bfloat16, tag=f"xq{b}", name=f"xq_d{b}") for b in range(NSUP)]
            p_d = [dram.tile([TT, IC], dt.float32, tag=f"p{t}", name=f"p_d{t}") for t in range(NT)]
            iq_d = [dram.tile([TSUP, ICP], dt.bfloat16, tag=f"iq{b}", name=f"iq_d{b}") for b in range(NSUP)]
            part_d = [dram.tile([TSUP, H], dt.float32, tag=f"part{b}", name=f"part_d{b}") for b in range(NSUP)]
            rs_d = [dram.tile([TSUP // N_CORES, H], dt.float32, tag=f"rs{b}", name=f"rs_d{b}")
                    for b in range(NSUP)]
            wsgu_d = dram.tile([1, 2], dt.float32, tag="wsgu_d")
            wsgu_a = dram.tile([1, 2], dt.float32, tag="wsgu_a")
            wsd_d = dram.tile([1, 1], dt.float32, tag="wsd_d")
            wsd_a = dram.tile([1, 1], dt.float32, tag="wsd_a")
            gpart_d = dram.tile([128, NT], dt.float32, tag="gpart_d")
            gall_d = dram.tile([128, NT], dt.float32, tag="gall_d")

            pwbig = ctx.enter_context(tc.tile_pool(name="wbig", bufs=1))
            psc16 = ctx.enter_context(tc.tile_pool(name="sc16", bufs=2))
            px = ctx.enter_context(tc.tile_pool(name="px", bufs=2))
            pxq = ctx.enter_context(tc.tile_pool(name="pxq", bufs=2))
            pstg = ctx.enter_context(tc.tile_pool(name="pstg", bufs=2))
            pbig16 = ctx.enter_context(tc.tile_pool(name="big16", bufs=2))
            pp = ctx.enter_context(tc.tile_pool(name="pp", bufs=2))
            pr = ctx.enter_context(tc.tile_pool(name="pr", bufs=3))
            piq = ctx.enter_context(tc.tile_pool(name="piq", bufs=2))
            psm = ctx.enter_context(tc.tile_pool(name="psm", bufs=1))
            pps = ctx.enter_context(tc.tile_pool(name="ps", bufs=8, space="PSUM"))

            # --- small persistent tiles ---
            scs = psm.tile([1, 3], dt.float32, tag="scs")
            nc.sync.dma_start(scs[:], sc_in.ap())
            sbc = psm.tile([128, 3], dt.float32, tag="sbc")
            nc.gpsimd.partition_broadcast(sbc[:], scs[:])
            wacc = psm.tile([128, 3], dt.float32, tag="wacc")
            nc.vector.memset(wacc[:], 0.0)
            beta_all = psm.tile([128, NT], dt.float32, tag="beta_all")
            gam_p = psm.tile([128, NT], dt.float32, tag="gam_p")

            # ============ |w| sums (in-place Abs; w re-streamed later) ======
            def w_abs_sum(win, rows, wi):
                cols = win.shape[1]
                for r0, rr in row_tiles(rows):
                    for c0 in range(0, cols, HH):
                        cc = min(HH, cols - c0)
                        wt = psc16.tile([128, HH], dt.float32, tag="sc16",
                                        name=f"wt{wi}_{r0}_{c0}")
                        nc.sync.dma_start(wt[:rr, :cc], win.ap()[r0:r0 + rr, c0:c0 + cc])
                        acc = pr.tile([128, 1], dt.float32, tag="acc",
                                      name=f"wacc{wi}_{r0}_{c0}")
                        nc.scalar.activation(wt[:rr, :cc], wt[:rr, :cc], Act.Abs,
                                             accum_out=acc[:rr, :])
                        nc.vector.tensor_tensor(
                            out=wacc[:rr, wi:wi + 1], in0=wacc[:rr, wi:wi + 1],
                            in1=acc[:rr, :], op=Alu.add)

            w_abs_sum(wg_in, IC, 0)
            w_abs_sum(wu_in, IC, 1)
            wred = psm.tile([128, 3], dt.float32, tag="wred")
            nc.gpsimd.partition_all_reduce(wred[:, 0:2], wacc[:, 0:2], channels=128,
                                           reduce_op=bass_isa.ReduceOp.add)
            nc.sync.dma_start(wsgu_d[:], wred[0:1, 0:2])
            nc.gpsimd.collective_compute("AllReduce", Alu.add, replica_groups=RG,
                                         ins=[wsgu_d.opt()], outs=[wsgu_a.opt()])
            w_abs_sum(wd_in, H, 2)
            nc.gpsimd.partition_all_reduce(wred[:, 2:3], wacc[:, 2:3], channels=128,
                                           reduce_op=bass_isa.ReduceOp.add)
            nc.sync.dma_start(wsd_d[:], wred[0:1, 2:3])
            nc.gpsimd.collective_compute("AllReduce", Alu.add, replica_groups=RG,
                                         ins=[wsd_d.opt()], outs=[wsd_a.opt()])

            # global w means: wmv (value), wrec (1/(wm+EPS)), both [128,3]
            wsb = psm.tile([1, 3], dt.float32, tag="wsb")
            nc.sync.dma_start(wsb[:, 0:2], wsgu_a[:])
            nc.sync.dma_start(wsb[:, 2:3], wsd_a[:])
            wsbc = psm.tile([128, 3], dt.float32, tag="wsbc")
            nc.gpsimd.partition_broadcast(wsbc[:, 0:2], wsb[:, 0:2])
            nc.gpsimd.partition_broadcast(wsbc[:, 2:3], wsb[:, 2:3])
            wmv = psm.tile([128, 3], dt.float32, tag="wmv")
            nc.vector.tensor_scalar(out=wmv[:], in0=wsbc[:], scalar1=1.0 / (I * H),
                                    scalar2=None, op0=Alu.mult)
            wmd = psm.tile([128, 3], dt.float32, tag="wmd")
            nc.vector.tensor_scalar(out=wmd[:], in0=wsbc[:], scalar1=1.0 / (I * H),
                                    scalar2=EPS, op0=Alu.mult, op1=Alu.add)
            wrec = psm.tile([128, 3], dt.float32, tag="wrec")
            nc.vector.reciprocal(wrec[:], wmd[:])

            # ============ ternarize weights -> bf16 DRAM ============
            def w_quant(win, rows, wi, wdst):
                cols = win.shape[1]
                for r0, rr in row_tiles(rows):
                    for c0 in range(0, cols, HH):
                        cc = min(HH, cols - c0)
                        wt = psc16.tile([128, HH], dt.float32, tag="sc16",
                                        name=f"wq{wi}_{r0}_{c0}")
                        nc.sync.dma_start(wt[:rr, :cc], win.ap()[r0:r0 + rr, c0:c0 + cc])
                        nc.scalar.activation(wt[:rr, :cc], wt[:rr, :cc], Act.Copy,
                                             bias=MAGIC, scale=wrec[:rr, wi:wi + 1])
                        nc.vector.tensor_scalar(out=wt[:rr, :cc], in0=wt[:rr, :cc],
                                                scalar1=MAGIC + 1.0, scalar2=MAGIC - 1.0,
                                                op0=Alu.min, op1=Alu.max)
                        wq = psc16.tile([128, HH], dt.bfloat16, tag="sc16b",
                                        name=f"wqo{wi}_{r0}_{c0}")
                        nc.vector.tensor_scalar(out=wq[:rr, :cc], in0=wt[:rr, :cc],
                                                scalar1=-MAGIC, scalar2=None, op0=Alu.add)
                        nc.sync.dma_start(wdst[r0:r0 + rr, c0:c0 + cc], wq[:rr, :cc])

            w_quant(wg_in, IC, 0, wqg_d)
            w_quant(wu_in, IC, 1, wqu_d)

            # gate/up resident fp8: [128, 2*HB*IC], col ((w*HB+hb)*IC + i)
            wq8 = pwbig.tile([128, 2 * HB * IC], dt.float8e4, tag="wbig")
            for wi, wsrc in enumerate([wqg_d, wqu_d]):
                for hb in range(HB):
                    stg = pstg.tile([128, IC], dt.bfloat16, tag="pstg")
                    nc.sync.dma_start_transpose(stg[:], wsrc[:, hb * 128:(hb + 1) * 128])
                    off = (wi * HB + hb) * IC
                    nc.vector.tensor_copy(wq8[:, off:off + IC], stg[:])

            w_quant(wd_in, H, 2, wqd_d)

            # ============ x int4 quant + mm1 + p, per super-block ============
            for b in range(NSUP):
                for ti in range(NTS):
                    t = b * NTS + ti
                    t0 = t * TT
                    xh = [px.tile([128, HH], dt.float32, tag="px", name=f"xh{t}_{_h}") for _h in range(2)]
                    ac = [pr.tile([128, 1], dt.float32, tag="acc", name=f"ac{t}_{_h}") for _h in range(2)]
                    for h in range(2):
                        nc.sync.dma_start(xh[h][:], x_in.ap()[t0:t0 + TT,
                                                              h * HH:(h + 1) * HH])
                        nc.vector.tensor_reduce(out=ac[h][:], in_=xh[h][:],
                                                axis=mybir.AxisListType.X, op=Alu.add,
                                                apply_absolute_value=True)
                    asum = pr.tile([128, 1], dt.float32, tag="asum")
                    nc.vector.tensor_tensor(out=asum[:], in0=ac[0][:], in1=ac[1][:],
                                            op=Alu.add)
                    nc.vector.tensor_scalar(out=beta_all[:, t:t + 1], in0=asum[:],
                                            scalar1=1.0 / H, scalar2=None, op0=Alu.mult)
                    dbe = pr.tile([128, 1], dt.float32, tag="dbe")
                    nc.vector.tensor_scalar(out=dbe[:], in0=asum[:], scalar1=1.0 / H,
                                            scalar2=EPS, op0=Alu.mult, op1=Alu.add)
                    rbe = pr.tile([128, 1], dt.float32, tag="rbe")
                    nc.vector.reciprocal(rbe[:], dbe[:])
                    sbe = pr.tile([128, 1], dt.float32, tag="sbe")
                    nc.vector.tensor_scalar(out=sbe[:], in0=rbe[:], scalar1=SQRT7,
                                            scalar2=None, op0=Alu.mult)
                    for h in range(2):
                        nc.scalar.activation(xh[h][:], xh[h][:], Act.Copy, bias=MAGIC,
                                             scale=sbe[:])
                        nc.vector.tensor_scalar(out=xh[h][:], in0=xh[h][:],
                                                scalar1=MAGIC + 7.0, scalar2=MAGIC - 8.0,
                                                op0=Alu.min, op1=Alu.max)
                        xqh = pxq.tile([128, HH], dt.bfloat16, tag="pxq")
                        nc.vector.tensor_scalar(out=xqh[:], in0=xh[h][:],
                                                scalar1=-MAGIC, scalar2=None, op0=Alu.add)
                        nc.sync.dma_start(xq_d[b][ti * TT:(ti + 1) * TT,
                                                  h * HH:(h + 1) * HH], xqh[:])

                # transpose-read + fp8 cast: xqT8 [128, HB*TSUP]
                xqT8 = pbig16.tile([128, HB * TSUP], dt.float8e4, tag="big16")
                for hb in range(HB):
                    stg = pstg.tile([128, TSUP], dt.bfloat16, tag="pstg")
                    nc.sync.dma_start_transpose(stg[:], xq_d[b][:, hb * 128:(hb + 1) * 128])
                    nc.vector.tensor_copy(xqT8[:, hb * TSUP:(hb + 1) * TSUP], stg[:])

                for ti in range(NTS):
                    t = b * NTS + ti
                    gps = [pps.tile([128, ICQ], dt.float32, tag="ps", name=f"gps{t}_{_q}") for _q in range(4)]
                    ups = [pps.tile([128, ICQ], dt.float32, tag="ps", name=f"ups{t}_{_q}") for _q in range(4)]
                    for k in range(HB):
                        lhs = xqT8[:, k * TSUP + ti * TT: k * TSUP + (ti + 1) * TT]
                        st, sp = (k == 0), (k == HB - 1)
                        for q in range(4):
                            rg = wq8[:, k * IC + q * ICQ: k * IC + (q + 1) * ICQ]
                            nc.tensor.matmul(gps[q][:], lhs, rg, start=st, stop=sp)
                        for q in range(4):
                            ru = wq8[:, (HB + k) * IC + q * ICQ:
                                     (HB + k) * IC + (q + 1) * ICQ]
                            nc.tensor.matmul(ups[q][:], lhs, ru, start=st, stop=sp)
                    pt = pp.tile([128, IC], dt.float32, tag="pp")
                    for q in range(4):
                        rt = pr.tile([128, ICQ], dt.float32, tag="rt",
                                      name=f"rt{t}_{q}")
                        nc.scalar.activation(rt[:], gps[q][:], Act.Relu)
                        nc.scalar.activation(rt[:], rt[:], Act.Square)
                        nc.vector.tensor_tensor(out=pt[:, q * ICQ:(q + 1) * ICQ],
                                                in0=rt[:], in1=ups[q][:], op=Alu.mult)
                    nc.vector.tensor_reduce(out=gam_p[:, t:t + 1], in_=pt[:],
                                            axis=mybir.AxisListType.X, op=Alu.max,
                                            apply_absolute_value=True)
                    nc.sync.dma_start(p_d[t][:], pt[:])

            # ============ gamma AllReduce(max) + per-token scales ============
            nc.sync.dma_start(gpart_d[:], gam_p[:])
            nc.gpsimd.collective_compute("AllReduce", Alu.max, replica_groups=RG,
                                         ins=[gpart_d.opt()], outs=[gall_d.opt()])
            gam = psm.tile([128, NT], dt.float32, tag="gam")
            nc.sync.dma_start(gam[:], gall_d[:])

            # C_t = beta^3 * (wm_u*s_u) * (wm_g*s_g)^2
            # s2_t = 127*C/(C*gam + EPS);  D_t = C*gam*(wm_d*s_d)/127
            cgg = psm.tile([128, 1], dt.float32, tag="cgg")
            nc.vector.tensor_tensor(out=cgg[:], in0=wmv[:, 0:1], in1=sbc[:, 0:1],
                                    op=Alu.mult)
            cuu = psm.tile([128, 1], dt.float32, tag="cuu")
            nc.vector.tensor_tensor(out=cuu[:], in0=wmv[:, 1:2], in1=sbc[:, 1:2],
                                    op=Alu.mult)
            cdd = psm.tile([128, 1], dt.float32, tag="cdd")
            nc.vector.tensor_tensor(out=cdd[:], in0=wmv[:, 2:3], in1=sbc[:, 2:3],
                                    op=Alu.mult)
            cb = psm.tile([128, 1], dt.float32, tag="cb")
            nc.vector.tensor_tensor(out=cb[:], in0=cgg[:], in1=cgg[:], op=Alu.mult)
            nc.vector.tensor_tensor(out=cb[:], in0=cb[:], in1=cuu[:], op=Alu.mult)
            b2 = psm.tile([128, NT], dt.float32, tag="b2")
            nc.vector.tensor_tensor(out=b2[:], in0=beta_all[:], in1=beta_all[:],
                                    op=Alu.mult)
            nc.vector.tensor_tensor(out=b2[:], in0=b2[:], in1=beta_all[:], op=Alu.mult)
            Ct = psm.tile([128, NT], dt.float32, tag="Ct")
            nc.vector.tensor_scalar(out=Ct[:], in0=b2[:], scalar1=cb[:], scalar2=None,
                                    op0=Alu.mult)
            cg = psm.tile([128, NT], dt.float32, tag="cg")
            nc.vector.tensor_tensor(out=cg[:], in0=Ct[:], in1=gam[:], op=Alu.mult)
            rn = psm.tile([128, NT], dt.float32, tag="rn")
            nc.vector.tensor_scalar(out=rn[:], in0=cg[:], scalar1=EPS, scalar2=None,
                                    op0=Alu.add)
            nc.vector.reciprocal(rn[:], rn[:])
            s2 = psm.tile([128, NT], dt.float32, tag="s2")
            nc.vector.tensor_scalar(out=s2[:], in0=Ct[:], scalar1=127.0, scalar2=None,
                                    op0=Alu.mult)
            nc.vector.tensor_tensor(out=s2[:], in0=s2[:], in1=rn[:], op=Alu.mult)
            Dt = psm.tile([128, NT], dt.float32, tag="Dt")
            nc.vector.tensor_scalar(out=Dt[:], in0=cg[:], scalar1=cdd[:], scalar2=None,
                                    op0=Alu.mult)
            nc.vector.tensor_scalar(out=Dt[:], in0=Dt[:], scalar1=1.0 / 127.0,
                                    scalar2=None, op0=Alu.mult)

            # ============ int8 quant + mm2 + ReduceScatter per super ========
            # w_down^T resident bf16: [128, NSLAB*H]; slab sb = i rows sb*128..,
            # pad i-rows zeroed.
            wdq = pwbig.tile([128, NSLAB * H], dt.bfloat16, tag="wbig")
            for sb in range(NSLAB):
                i0 = sb * 128
                rr = min(128, IC - i0)
                nc.sync.dma_start_transpose(wdq[:rr, sb * H:(sb + 1) * H],
                                            wqd_d[:, i0:i0 + rr])
                if rr < 128:
                    nc.vector.memset(wdq[rr:128, sb * H:(sb + 1) * H], 0.0)

            for b in range(NSUP):
                for ti in range(NTS):
                    t = b * NTS + ti
                    pt = pp.tile([128, IC], dt.float32, tag="pp")
                    nc.sync.dma_start(pt[:], p_d[t][:])
                    nc.scalar.activation(pt[:], pt[:], Act.Copy, bias=MAGIC,
                                         scale=s2[:, t:t + 1])
                    nc.vector.tensor_scalar(out=pt[:], in0=pt[:],
                                            scalar1=MAGIC + 127.0,
                                            scalar2=MAGIC - 128.0,
                                            op0=Alu.min, op1=Alu.max)
                    qt = piq.tile([128, ICP], dt.bfloat16, tag="piq")
                    nc.vector.tensor_scalar(out=qt[:, 0:IC], in0=pt[:], scalar1=-MAGIC,
                                            scalar2=None, op0=Alu.add)
                    nc.vector.memset(qt[:, IC:ICP], 0.0)
                    nc.sync.dma_start(iq_d[b][ti * TT:(ti + 1) * TT, :], qt[:])

                iqT = pbig16.tile([128, NSLAB * TSUP], dt.bfloat16, tag="big16")
                for sb in range(NSLAB):
                    nc.sync.dma_start_transpose(iqT[:, sb * TSUP:(sb + 1) * TSUP],
                                                iq_d[b][:, sb * 128:(sb + 1) * 128])

                for ti in range(NTS):
                    t = b * NTS + ti
                    ops = [pps.tile([128, 512], dt.float32, tag="ps", name=f"ops{t}_{_h}") for _h in range(8)]
                    for k in range(NSLAB):
                        lhs = iqT[:, k * TSUP + ti * TT: k * TSUP + (ti + 1) * TT]
                        st, sp = (k == 0), (k == NSLAB - 1)
                        for hh in range(8):
                            rh = wdq[:, k * H + hh * 512: k * H + (hh + 1) * 512]
                            nc.tensor.matmul(ops[hh][:], lhs, rh, start=st, stop=sp)
                    for half in range(2):
                        ev = psc16.tile([128, HH], dt.float32, tag="sc16",
                                        name=f"ev{t}_{half}")
                        for hh in range(4):
                            g = half * 4 + hh
                            nc.scalar.activation(ev[:, hh * 512:(hh + 1) * 512],
                                                 ops[g][:], Act.Copy,
                                                 scale=Dt[:, t:t + 1])
                        nc.sync.dma_start(part_d[b][ti * TT:(ti + 1) * TT,
                                                    half * HH:(half + 1) * HH], ev[:])

                nc.gpsimd.collective_compute("ReduceScatter", Alu.add,
                                             replica_groups=RG,
                                             ins=[part_d[b].opt()],
                                             outs=[rs_d[b].opt()])
                nc.sync.dma_start(out_ext.ap()[b], rs_d[b][:])

            if debug:
                nc.sync.dma_start(dbg["dbg_beta"].ap(), beta_all[:])
                nc.sync.dma_start(dbg["dbg_gam"].ap(), gam[:])
                nc.sync.dma_start(dbg["dbg_s2"].ap(), s2[:])
                nc.sync.dma_start(dbg["dbg_Dt"].ap(), Dt[:])
                nc.sync.dma_start(dbg["dbg_wm"].ap(), wmv[0:1, :])
                nc.sync.dma_start(dbg["dbg_xq"].ap(), xq_d[0][:])
                nc.sync.dma_start(dbg["dbg_wqg"].ap(), wqg_d[:])
                nc.sync.dma_start(dbg["dbg_p"].ap(), p_d[0][:])
                nc.sync.dma_start(dbg["dbg_iq"].ap(), iq_d[0][:])
                nc.sync.dma_start(dbg["dbg_part"].ap(), part_d[0][:])

    nc.compile()
    return nc


def _get_compiled():
    if "nc" not in _cache:
        _cache["nc"] = _build()
    return _cache["nc"]


def kernel(x, w_gate, w_up, w_down, s_gate, s_up, s_down):
    from concourse.bass_utils import run_bass_kernel_spmd

    nc = _get_compiled()

    xf = np.ascontiguousarray(np.asarray(x).reshape(T, H).astype(np.float32,
                                                                 copy=False))
    scales = np.array([[float(np.asarray(s_gate).reshape(-1)[0]),
                        float(np.asarray(s_up).reshape(-1)[0]),
                        float(np.asarray(s_down).reshape(-1)[0])]],
                      dtype=np.float32)
    in_maps = []
    for c in range(N_CORES):
        i0 = c * IC
        in_maps.append({
            "x": xf,
            "wg": np.ascontiguousarray(w_gate[i0:i0 + IC, :], dtype=np.float32),
            "wu": np.ascontiguousarray(w_up[i0:i0 + IC, :], dtype=np.float32),
            "wd": np.ascontiguousarray(w_down[:, i0:i0 + IC], dtype=np.float32),
            "scales": scales,
        })

    res = run_bass_kernel_spmd(nc, in_maps, core_ids=list(range(N_CORES)))

    out = np.empty((T, H), dtype=np.float32)
    tpc = TSUP // N_CORES
    for c in range(N_CORES):
        o = res.results[c]["out_rs"]
        for b in range(NSUP):
            out[b * TSUP + c * tpc: b * TSUP + (c + 1) * tpc] = o[b]
    return out.reshape(B, S, H)
